# revision 14
# baseline (speedup 1.0000x reference)
"""MiniRocket feature kernel for Trainium2 (8 NeuronCores, batch-parallel).

Math (per batch example b, dilation i with d in (1,2,4,8), pad p=4d):
  conv[c,j,t] = sum_k base[j,k] * x_pad[c, t + k*d]          (zero pad p)
  csum[j,t]   = sum_c comb[i,j,c] * conv[c,j,t]
  sig[j,t,f]  = sigmoid(csum[j,t] - bias[i,j,f])
  feat        = mean_t sig  (full range if (i+j)%2==0 else interior [p, L-p))

Everything up to the sigmoid is linear in x, so for each output triple
q=(i,j,f) there is one fused weight vector over (tap k, channel c):
  W[(k,c), q] = base[j,k] * comb[i,j,c]
and csum[q,t] = sum_{k,c} W[(k,c), q] * R_i[(k,c), t] with
  R_i[(k,c), t] = x_pad[c, t + k*d - p].

Hardware mapping per core (one batch example):
  - 10080 triples q padded to 79 ops x 128 partitions.
  - PE: per op, 4 matmuls (K=72, N=512, float32r) -> PSUM (128, 2048).
  - ACT: one sigmoid over (128, 2048) with per-partition bias and
    accum_out = per-partition sum over t (the full-range sum, free).
  - DVE: tiny reduces over the p edge columns for the trimmed mean.
  - Epilogue: feats = A*full_sum + Bk*(eL+eR), DMA out (128, 79).
Host reorders (o,p) -> q and stacks the 8 per-core rows.
"""

import ml_dtypes
import numpy as np

from concourse import bacc, bass, bass_utils, tile
from concourse import mybir

B, C, L = 8, 8, 2048
DILS = (1, 2, 4, 8)
NK, NF, NT = 84, 30, 9  # kernels, features-per-dilation, taps
QD = NK * NF            # triples per dilation (2520)
Q = len(DILS) * QD      # 10080
NOPS = (Q + 127) // 128  # 79
QPAD = NOPS * 128        # 10112

F32 = mybir.dt.float32
F32R = mybir.dt.float32r
BF16 = mybir.dt.bfloat16


def _segments(o):
    """Partition rows of op o grouped by dilation: [(r0, r1, i), ...]."""
    q0, q1 = o * 128, min(o * 128 + 128, Q)
    segs = []
    qa = q0
    while qa < q1:
        i = qa // QD
        qe = min((i + 1) * QD, q1)
        segs.append((qa - q0, qe - q0, i))
        qa = qe
    return segs


_BOUNDARY_OPS = [o for o in range(NOPS) if len(_segments(o)) > 1]  # [19, 39, 59]


def _build_module():
    nc = bacc.Bacc("TRN2", target_bir_lowering=False, debug=False, num_devices=8)

    X = nc.dram_tensor("x", [C, L], BF16, kind="ExternalInput")
    WALL = nc.dram_tensor("wall", [NT * C, QPAD], BF16, kind="ExternalInput")
    WB = nc.dram_tensor("wb", [2 * len(_BOUNDARY_OPS), NT * C, 128], BF16,
                        kind="ExternalInput")
    BIASP = nc.dram_tensor("biasp", [128, NOPS], F32, kind="ExternalInput")
    APK = nc.dram_tensor("apack", [128, NOPS], F32, kind="ExternalInput")
    BPK = nc.dram_tensor("bpack", [128, NOPS], F32, kind="ExternalInput")
    OUT = nc.dram_tensor("out", [128, NOPS], F32, kind="ExternalOutput")

    with tile.TileContext(nc) as tc:
        with tc.tile_pool(name="const", bufs=1) as cp, \
             tc.tile_pool(name="sig", bufs=3) as sp, \
             tc.tile_pool(name="ps", bufs=2, space="PSUM") as pp:

            # ---- constants / input staging ----
            # DMA issue costs ~0.7us of engine time per dma_start, so the
            # prologue uses few, large DMAs spread over three idle engines.
            xpad = cp.tile([C, L + 64], BF16)
            nc.gpsimd.memset(xpad[:], 0.0)
            nc.sync.dma_start(out=xpad[:, 32:32 + L], in_=X[:])

            # preload the sigmoid table set (~2.7us) off the critical path
            tgt = cp.tile([128, 1], F32)
            tdum = cp.tile([128, 1], F32)
            nc.vector.memset(tdum[:], 0.0)
            nc.scalar.activation(tgt[:], tdum[:],
                                 mybir.ActivationFunctionType.Sigmoid)

            biasp = cp.tile([128, NOPS], F32)
            nc.sync.dma_start(out=biasp[:], in_=BIASP[:])

            wall = cp.tile([NT * C, QPAD], BF16)
            CHUNK = 1408  # 11 ops per DMA chunk so op 0 isn't gated on all of W
            for c0 in range(0, QPAD, CHUNK):
                c1 = min(c0 + CHUNK, QPAD)
                nc.sync.dma_start(out=wall[:, c0:c1], in_=WALL[:, c0:c1])

            wbs = []
            for m in range(2 * len(_BOUNDARY_OPS)):
                t = cp.tile([NT * C, 128], BF16, name=f"wb{m}")
                nc.gpsimd.dma_start(out=t[:], in_=WB[m])
                wbs.append(t)

            apk = cp.tile([128, NOPS], F32)
            nc.gpsimd.dma_start(out=apk[:], in_=APK[:])
            bpk = cp.tile([128, NOPS], F32)
            nc.gpsimd.dma_start(out=bpk[:], in_=BPK[:])

            # ---- build the 4 shifted-tap matrices R_i (72, 2048) ----
            # One DMA per dilation: a 3D windowed access pattern reads the
            # 9 overlapping tap windows directly (k index is c-major:
            # row c*9+k of R_i holds x_pad[c, t + k*d - p]).
            Rs = []
            for i, d in enumerate(DILS):
                R = cp.tile([NT * C, L], BF16, name=f"R{i}")
                off0 = 32 - 4 * d
                src = xpad[:, off0:off0 + L]
                dims = src.ap
                dims.clear()
                dims.append((L + 64, C))
                dims.append((d, NT))
                dims.append((1, L))
                nc.gpsimd.dma_start(out=R[:], in_=src)
                Rs.append(R)

            # ---- accumulators ----
            acc = cp.tile([128, NOPS], F32)
            eL = cp.tile([128, NOPS], F32)
            eR = cp.tile([128, NOPS], F32)
            nc.gpsimd.memset(acc[:], 0.0)
            nc.gpsimd.memset(eL[:], 0.0)
            nc.gpsimd.memset(eR[:], 0.0)

            # ---- main loop over the 79 partition-ops ----
            for o in range(NOPS):
                segs = _segments(o)
                ps = pp.tile([128, L], F32, tag="ps", name="ps")
                if len(segs) == 1:
                    i = segs[0][2]
                    for c in range(4):
                        nc.tensor.matmul(
                            ps[:, c * 512:(c + 1) * 512],
                            wall[:, o * 128:(o + 1) * 128],
                            Rs[i][:, c * 512:(c + 1) * 512],
                            start=True, stop=True)
                else:
                    m = _BOUNDARY_OPS.index(o)
                    for c in range(4):
                        nc.tensor.matmul(
                            ps[:, c * 512:(c + 1) * 512],
                            wbs[2 * m],
                            Rs[segs[0][2]][:, c * 512:(c + 1) * 512],
                            start=True, stop=False)
                        nc.tensor.matmul(
                            ps[:, c * 512:(c + 1) * 512],
                            wbs[2 * m + 1],
                            Rs[segs[1][2]][:, c * 512:(c + 1) * 512],
                            start=False, stop=True)

                sig = sp.tile([128, L], F32, tag="sig", name="sig")
                nc.scalar.activation(
                    sig[:], ps[:],
                    mybir.ActivationFunctionType.Sigmoid,
                    bias=biasp[:, o:o + 1],
                    accum_out=acc[:, o:o + 1])

                # Engine APs with a non-zero partition start are restricted
                # (verifier: start must be 32-aligned AND span <= 32), so
                # every reduce starts at partition 0 and spans [0, r1). For
                # boundary ops, emit segments in reverse: the wide reduce
                # (second dilation's width) goes first, then the first
                # segment's reduce overwrites the prefix with correct values.
                for (r0, r1, i) in reversed(segs):
                    p = 4 * DILS[i]
                    nc.vector.reduce_sum(eL[0:r1, o:o + 1], sig[0:r1, 0:p],
                                         axis=mybir.AxisListType.X)
                    nc.vector.reduce_sum(eR[0:r1, o:o + 1], sig[0:r1, L - p:L],
                                         axis=mybir.AxisListType.X)

            # ---- epilogue: feats = apk*acc + bpk*(eL+eR) ----
            e = cp.tile([128, NOPS], F32)
            nc.vector.tensor_add(e[:], eL[:], eR[:])
            t0 = cp.tile([128, NOPS], F32)
            nc.vector.tensor_mul(t0[:], acc[:], apk[:])
            # t0 + bpk*e in one fused op: (e * bpk) + t0
            feats = cp.tile([128, NOPS], F32)
            nc.vector.tensor_mul(e[:], e[:], bpk[:])
            nc.vector.tensor_add(feats[:], t0[:], e[:])
            nc.sync.dma_start(out=OUT[:], in_=feats[:])

    nc.compile()
    return nc


def _host_constants(kernels, comb, biases):
    """Build the fused weight/bias/scale tables shared by all cores."""
    base = np.asarray(kernels, np.float32).reshape(-1, NT)[:NK]  # (84, 9)
    comb = np.asarray(comb, np.float32)     # (4, 84, 8)
    biases = np.asarray(biases, np.float32)  # (4, 84, 30)

    qs = np.arange(QPAD)
    valid = qs < Q
    ii = np.minimum(qs // QD, len(DILS) - 1)
    jj = (qs % QD) // NF
    ff = qs % NF

    bq = base[jj]            # (QPAD, 9)
    cq = comb[ii, jj]        # (QPAD, 8)
    # k index is c-major (k = c*9 + ktap) to match the windowed R DMA
    wall = (cq[:, :, None] * bq[:, None, :]).reshape(QPAD, NT * C)
    wall = (wall * valid[:, None]).T.astype(np.float32).copy()  # (72, QPAD)

    wb = np.zeros((2 * len(_BOUNDARY_OPS), NT * C, 128), np.float32)
    for m, o in enumerate(_BOUNDARY_OPS):
        cols = wall[:, o * 128:(o + 1) * 128]
        seg_i = ii[o * 128:(o + 1) * 128]
        segs = _segments(o)
        wb[2 * m] = cols * (seg_i == segs[0][2])
        wb[2 * m + 1] = cols * (seg_i == segs[1][2])

    biasp = np.zeros((128, NOPS), np.float32)
    apack = np.zeros((128, NOPS), np.float32)
    bpack = np.zeros((128, NOPS), np.float32)
    bias_q = -biases[ii, jj, ff] * valid
    parity = ((ii + jj) % 2 == 0)
    p_q = 4 * np.asarray(DILS)[ii]
    a_q = np.where(parity, 1.0 / L, 1.0 / (L - 2 * p_q)) * valid
    b_q = np.where(parity, 0.0, -1.0 / (L - 2 * p_q)) * valid
    biasp[qs % 128, qs // 128] = bias_q
    apack[qs % 128, qs // 128] = a_q
    bpack[qs % 128, qs // 128] = b_q
    return wall, wb, biasp, apack, bpack


_NC = None


def _get_module():
    global _NC
    if _NC is None:
        _NC = _build_module()
    return _NC


def run(inputs, trace=False, **trace_kwargs):
    """Run on 8 cores; returns (out (8, 10080) f32, BassKernelResults)."""
    x = np.ascontiguousarray(np.asarray(inputs["x"], np.float32))
    wall, wb, biasp, apack, bpack = _host_constants(
        inputs["kernels"], inputs["comb"], inputs["biases"])

    nc = _get_module()
    bf = ml_dtypes.bfloat16
    wall_b = wall.astype(bf)
    wb_b = wb.astype(bf)
    in_maps = []
    for b in range(B):
        in_maps.append({
            "x": np.ascontiguousarray(x[b]).astype(bf),
            "wall": wall_b, "wb": wb_b, "biasp": biasp,
            "apack": apack, "bpack": bpack,
        })
    res = bass_utils.run_bass_kernel_spmd(
        nc, in_maps, core_ids=list(range(B)), trace=trace, **trace_kwargs)

    out = np.empty((B, Q), np.float32)
    for b in range(B):
        r = res.results[b]["out"]          # (128, 79)
        out[b] = r.T.reshape(-1)[:Q]       # q = o*128 + p
    return out, res


def kernel(x, kernels, comb, biases):
    out, _ = run({"x": x, "kernels": kernels, "comb": comb, "biases": biases})
    return out


# revision 17
# speedup vs baseline: 1.0885x; 1.0885x over previous
"""MiniRocket feature kernel for Trainium2 (8 NeuronCores, batch-parallel).

Math (per batch example b, dilation i with d in (1,2,4,8), pad p=4d):
  conv[c,j,t] = sum_k base[j,k] * x_pad[c, t + k*d]          (zero pad p)
  csum[j,t]   = sum_c comb[i,j,c] * conv[c,j,t]
  sig[j,t,f]  = sigmoid(csum[j,t] - bias[i,j,f])
  feat        = mean_t sig  (full range if (i+j)%2==0 else interior [p, L-p))

Everything up to the sigmoid is linear in x, so for each output triple
q=(i,j,f) there is one fused weight vector over (channel c, tap k):
  W[(c,k), q] = base[j,k] * comb[i,j,c]
and csum[q,t] = sum_{c,k} W[(c,k), q] * R_i[(c,k), t] with
  R_i[(c,k), t] = x_pad[c, t + k*d - p].

Hardware mapping per core (one batch example):
  - triples grouped per dilation into 20 ops x 128 partitions (2520
    triples + 40 pad rows per dilation) -> 80 uniform ops total.
  - R_i (72, 2048) built by ONE windowed 3D-AP DMA from the host-padded
    DRAM x_pad (the 9 overlapping tap windows are strides, not copies).
  - PE: per op, 4 matmuls (K=72, N=512, bf16) -> PSUM (128, 2048) f32.
  - ACT: one sigmoid over (128, 2048) with per-partition bias and
    accum_out = per-partition sum over t (the full-range sum, free).
  - DVE: tiny reduces over the p edge columns for the trimmed mean.
  - Epilogue: feats = A*full_sum + Bk*(eL+eR), DMA out (128, 80).
Host reorders (op, partition) -> q and stacks the 8 per-core rows.
"""

import ml_dtypes
import numpy as np

from concourse import bacc, bass, bass_utils, tile
from concourse import mybir

B, C, L = 8, 8, 2048
DILS = (1, 2, 4, 8)
ND = len(DILS)
NK, NF, NT = 84, 30, 9   # kernels, features-per-dilation, taps
QD = NK * NF             # triples per dilation (2520)
Q = ND * QD              # 10080
OPD = (QD + 127) // 128  # ops per dilation (20)
NOPS = ND * OPD          # 80
QDP = OPD * 128          # padded triples per dilation (2560)
QPAD = ND * QDP          # 10240
PADW = 32                # host-side zero pad columns each side of x

F32 = mybir.dt.float32
BF16 = mybir.dt.bfloat16


def _build_module():
    nc = bacc.Bacc("TRN2", target_bir_lowering=False, debug=False, num_devices=8)

    XPAD = nc.dram_tensor("xpad", [C, L + 2 * PADW], BF16, kind="ExternalInput")
    WALL = nc.dram_tensor("wall", [NT * C, QPAD], BF16, kind="ExternalInput")
    BIASP = nc.dram_tensor("biasp", [128, NOPS], F32, kind="ExternalInput")
    APK = nc.dram_tensor("apack", [128, NOPS], F32, kind="ExternalInput")
    BPK = nc.dram_tensor("bpack", [128, NOPS], F32, kind="ExternalInput")
    OUT = nc.dram_tensor("out", [128, NOPS], F32, kind="ExternalOutput")

    with tile.TileContext(nc) as tc:
        with tc.tile_pool(name="const", bufs=1) as cp, \
             tc.tile_pool(name="sig", bufs=3) as sp, \
             tc.tile_pool(name="ps", bufs=2, space="PSUM") as pp:

            # preload the sigmoid table set (~2.7us) off the critical path
            tgt = cp.tile([128, 1], F32)
            tdum = cp.tile([128, 1], F32)
            nc.vector.memset(tdum[:], 0.0)
            nc.scalar.activation(tgt[:], tdum[:],
                                 mybir.ActivationFunctionType.Sigmoid)

            # ---- R_i (72, 2048): one windowed DMA per dilation from the
            # host-padded DRAM x. Row c*9+k holds x_pad[c, t + k*d - 4d]
            # (c-major k to match the DMA's flat iteration order).
            Rs = []
            for i, d in enumerate(DILS):
                R = cp.tile([NT * C, L], BF16, name=f"R{i}")
                src = XPAD[:, PADW - 4 * d:PADW - 4 * d + L]
                dims = src.ap
                dims.clear()
                dims.append((L + 2 * PADW, C))
                dims.append((d, NT))
                dims.append((1, L))
                eng = nc.gpsimd if i % 2 == 0 else nc.scalar
                eng.dma_start(out=R[:], in_=src)
                Rs.append(R)

            # ---- shared constants ----
            biasp = cp.tile([128, NOPS], F32)
            nc.sync.dma_start(out=biasp[:], in_=BIASP[:])

            wall = cp.tile([NT * C, QPAD], BF16)
            CHUNK = 1280  # 10 ops per DMA chunk so op 0 isn't gated on all of W
            for c0 in range(0, QPAD, CHUNK):
                c1 = min(c0 + CHUNK, QPAD)
                nc.sync.dma_start(out=wall[:, c0:c1], in_=WALL[:, c0:c1])

            apk = cp.tile([128, NOPS], F32)
            nc.gpsimd.dma_start(out=apk[:], in_=APK[:])
            bpk = cp.tile([128, NOPS], F32)
            nc.gpsimd.dma_start(out=bpk[:], in_=BPK[:])

            # ---- accumulators ----
            acc = cp.tile([128, NOPS], F32)
            eL = cp.tile([128, NOPS], F32)
            eR = cp.tile([128, NOPS], F32)
            nc.gpsimd.memset(acc[:], 0.0)
            nc.gpsimd.memset(eL[:], 0.0)
            nc.gpsimd.memset(eR[:], 0.0)

            # ---- main loop: 80 uniform ops (20 per dilation) ----
            for o in range(NOPS):
                i = o // OPD
                p = 4 * DILS[i]
                ps = pp.tile([128, L], F32, tag="ps", name="ps")
                for c in range(4):
                    nc.tensor.matmul(
                        ps[:, c * 512:(c + 1) * 512],
                        wall[:, o * 128:(o + 1) * 128],
                        Rs[i][:, c * 512:(c + 1) * 512],
                        start=True, stop=True)

                sig = sp.tile([128, L], F32, tag="sig", name="sig")
                nc.scalar.activation(
                    sig[:], ps[:],
                    mybir.ActivationFunctionType.Sigmoid,
                    bias=biasp[:, o:o + 1],
                    accum_out=acc[:, o:o + 1])

                # pad rows (last op per dilation) produce junk edge sums;
                # bpack=0 there zeroes them in the epilogue
                nc.vector.reduce_sum(eL[:, o:o + 1], sig[:, 0:p],
                                     axis=mybir.AxisListType.X)
                nc.vector.reduce_sum(eR[:, o:o + 1], sig[:, L - p:L],
                                     axis=mybir.AxisListType.X)

            # ---- epilogue: feats = apk*acc + bpk*(eL+eR) ----
            e = cp.tile([128, NOPS], F32)
            nc.vector.tensor_add(e[:], eL[:], eR[:])
            t0 = cp.tile([128, NOPS], F32)
            nc.vector.tensor_mul(t0[:], acc[:], apk[:])
            feats = cp.tile([128, NOPS], F32)
            nc.vector.tensor_mul(e[:], e[:], bpk[:])
            nc.vector.tensor_add(feats[:], t0[:], e[:])
            nc.sync.dma_start(out=OUT[:], in_=feats[:])

    nc.compile()
    return nc


def _host_constants(kernels, comb, biases):
    """Build the fused weight/bias/scale tables shared by all cores."""
    base = np.asarray(kernels, np.float32).reshape(-1, NT)[:NK]  # (84, 9)
    comb = np.asarray(comb, np.float32)      # (4, 84, 8)
    biases = np.asarray(biases, np.float32)  # (4, 84, 30)

    qs = np.arange(QPAD)
    ii = qs // QDP
    rr = qs % QDP                 # padded within-dilation index
    valid = rr < QD
    jj = np.minimum(rr, QD - 1) // NF
    ff = rr % NF

    bq = base[jj]            # (QPAD, 9)
    cq = comb[ii, jj]        # (QPAD, 8)
    # k index is c-major (k = c*9 + ktap) to match the windowed R DMA
    wall = (cq[:, :, None] * bq[:, None, :]).reshape(QPAD, NT * C)
    wall = (wall * valid[:, None]).T.astype(np.float32).copy()  # (72, QPAD)

    biasp = np.zeros((128, NOPS), np.float32)
    apack = np.zeros((128, NOPS), np.float32)
    bpack = np.zeros((128, NOPS), np.float32)
    bias_q = -biases[ii, jj, ff] * valid
    parity = ((ii + jj) % 2 == 0)
    p_q = 4 * np.asarray(DILS)[ii]
    a_q = np.where(parity, 1.0 / L, 1.0 / (L - 2 * p_q)) * valid
    b_q = np.where(parity, 0.0, -1.0 / (L - 2 * p_q)) * valid
    biasp[qs % 128, qs // 128] = bias_q
    apack[qs % 128, qs // 128] = a_q
    bpack[qs % 128, qs // 128] = b_q
    return wall, biasp, apack, bpack


_NC = None


def _get_module():
    global _NC
    if _NC is None:
        _NC = _build_module()
    return _NC


def run(inputs, trace=False, **trace_kwargs):
    """Run on 8 cores; returns (out (8, 10080) f32, BassKernelResults)."""
    x = np.ascontiguousarray(np.asarray(inputs["x"], np.float32))
    wall, biasp, apack, bpack = _host_constants(
        inputs["kernels"], inputs["comb"], inputs["biases"])

    nc = _get_module()
    bf = ml_dtypes.bfloat16
    wall_b = wall.astype(bf)
    xpad = np.zeros((B, C, L + 2 * PADW), np.float32)
    xpad[:, :, PADW:PADW + L] = x
    xpad_b = xpad.astype(bf)
    in_maps = []
    for b in range(B):
        in_maps.append({
            "xpad": np.ascontiguousarray(xpad_b[b]),
            "wall": wall_b, "biasp": biasp,
            "apack": apack, "bpack": bpack,
        })
    res = bass_utils.run_bass_kernel_spmd(
        nc, in_maps, core_ids=list(range(B)), trace=trace, **trace_kwargs)

    out = np.empty((B, Q), np.float32)
    for b in range(B):
        r = res.results[b]["out"]                  # (128, 80)
        flat = r.T.reshape(-1)                     # padded q = o*128 + p
        out[b] = flat.reshape(ND, QDP)[:, :QD].reshape(-1)
    return out, res


def kernel(x, kernels, comb, biases):
    out, _ = run({"x": x, "kernels": kernels, "comb": comb, "biases": biases})
    return out
